# revision 1
# baseline (speedup 1.0000x reference)
"""Trainium2 Bass kernel for nn_MultiHeadedAttention (B=2,S=2048,D=1024,H=16).

Sharding: tensor-parallel over heads — 2 heads per core x 8 cores.
Each core computes its 2 heads' attention and a partial output projection
(y_partial [B*S, D]); the host sums the 8 partials and adds bo.

Device pipeline per core (all matmul operands bf16, fp32 PSUM accumulate):
  qT/kT = W @ xT (feature-major), v = x @ WvT (token-major)
  S^T[k,q] = K @ Q^T (row-packed 2 heads, contraction dk=64)
  p = exp(S^T) * expbm   (expbm = exp(bias)*(1-mask), host-precomputed, bf16)
  OT = V^T @ p (col-packed 2 heads) ; d = ones^T @ p (partition-broadcast)
  OTn = OT * recip(d) -> y_partial = OTn^T @ WoT

Batch-1 projections are emitted interleaved into batch-0's attention so the
TensorE stream stays dense (HAM stays at K=8/8).
"""

import numpy as np
import ml_dtypes

import concourse.bass as bass
import concourse.tile as tile
from concourse import bacc, mybir
from concourse.bass import ts
from concourse.bass_utils import run_bass_kernel_spmd
from concourse.masks import make_identity

BF16 = ml_dtypes.bfloat16

B, S, D, H = 2, 2048, 1024, 16
N_CORES = 8
HC = H // N_CORES          # heads per core = 2
DK = D // H                # 64
DKC = HC * DK              # head dims per core = 128
P = 128
T = B * S                  # 4096 tokens
KO = D // P                # 8 feature k-subtiles
TC = 512                   # token chunk for projections
QC = 1024                  # q chunk for attention phase
NKS = S // P               # 16 k-subtiles per batch
NQC = S // QC              # 2 q-chunks per batch

bf = mybir.dt.bfloat16
f32 = mybir.dt.float32


class _Ctx:
    pass


def _proj_chunk(nc, g, c):
    """Emit projection work for token chunk c (512 tokens)."""
    io = g.io
    # v first, then k/q (dense chains) to leave PE warm.
    # v is computed feature-major (dense N=512 matmuls), then PE-transposed
    # into the token-major layout PV needs.
    xv = g.stream_pool.tile([P, KO, TC], bf, tag="xv", bufs=4, name=f"xv{c}")
    nc.sync.dma_start(
        xv[:], io["xvT"].rearrange("(ko ki) t -> ki ko t", ki=P)[:, :, ts(c, TC)]
    )
    ps_vT = g.psum_pool.tile([P, QC], f32, tag="ps_s", bufs=2, name=f"psvT{c}")
    for ko in range(KO):
        nc.tensor.matmul(
            ps_vT[:, :TC], g.wv_sb[:, ko, :], xv[:, ko, :],
            start=(ko == 0), stop=(ko == KO - 1),
        )
    vt_tmp = g.work_pool.tile([P, TC], bf, tag="vtt", bufs=2, name=f"vtt{c}")
    nc.vector.tensor_copy(vt_tmp[:], ps_vT[:, :TC])
    for tt in range(TC // P):
        ps_t = g.psum_pool.tile([P, P], bf, tag="ps_s", bufs=2, name=f"pst{c}_{tt}")
        nc.tensor.transpose(ps_t[:], vt_tmp[:, ts(tt, P)], g.ident_sb[:])
        vt_i = c * (TC // P) + tt
        nc.vector.tensor_copy(g.v_sb[:, vt_i, 0:DK], ps_t[:, 0:DK])
        nc.vector.tensor_copy(g.v_sb[:, vt_i, DK + 1:DKC + 1], ps_t[:, DK:DKC])

    xk = g.stream_pool.tile([P, KO, TC], bf, tag="xk", bufs=4, name=f"xk{c}")
    nc.sync.dma_start(
        xk[:], io["xkT"].rearrange("(ko ki) t -> ki ko t", ki=P)[:, :, ts(c, TC)]
    )
    ps_k = g.psum_pool.tile([P, QC], f32, tag="ps_s", bufs=2, name=f"psk{c}")
    for ko in range(KO):
        nc.tensor.matmul(
            ps_k[:, :TC], g.wk_sb[:, ko, :], xk[:, ko, :],
            start=(ko == 0), stop=(ko == KO - 1),
        )
    nc.vector.tensor_copy(g.kT_sb[:, ts(c, TC)], ps_k[:, :TC])

    xq = g.stream_pool.tile([P, KO, TC], bf, tag="xq", bufs=4, name=f"xq{c}")
    nc.sync.dma_start(
        xq[:], io["xqT"].rearrange("(ko ki) t -> ki ko t", ki=P)[:, :, ts(c, TC)]
    )
    ps_q = g.psum_pool.tile([P, QC], f32, tag="ps_s", bufs=2, name=f"psq{c}")
    for ko in range(KO):
        nc.tensor.matmul(
            ps_q[:, :TC], g.wq_sb[:, ko, :], xq[:, ko, :],
            start=(ko == 0), stop=(ko == KO - 1),
        )
    nc.vector.tensor_copy(g.qT_sb[:, ts(c, TC)], ps_q[:, :TC])


def _attention_chunk(nc, g, b, qc):
    """Emit attention + output projection for (batch b, q-chunk qc)."""
    io = g.io
    qs = b * S + qc * QC
    ps_oA = g.psum_pool.tile([P, QC], f32, tag="ps_o", bufs=1, name=f"psoA{b}_{qc}")
    ps_oB = g.psum_pool.tile([P, QC], f32, tag="ps_d", bufs=1, name=f"psoB{b}_{qc}")
    for ks in range(NKS):
        kslice = b * S + ks * P
        vt = kslice // P
        # scores^T for both heads (row-packed, K=64)
        ps_sA = g.psum_pool.tile([P, QC], f32, tag="ps_s", bufs=2,
                                 name=f"pssA{b}_{qc}_{ks}")
        ps_sB = g.psum_pool.tile([P, QC], f32, tag="ps_s", bufs=2,
                                 name=f"pssB{b}_{qc}_{ks}")
        # same lhsT for both chunks of one head -> adjacent emission
        for ch in range(QC // 512):
            nc.tensor.matmul(
                ps_sA[:, ts(ch, 512)],
                g.kT_sb[0:DK, kslice:kslice + P],
                g.qT_sb[0:DK, qs + ch * 512:qs + (ch + 1) * 512],
                start=True, stop=True,
            )
        for ch in range(QC // 512):
            nc.tensor.matmul(
                ps_sB[:, ts(ch, 512)],
                g.kT_sb[DK:P, kslice:kslice + P],
                g.qT_sb[DK:P, qs + ch * 512:qs + (ch + 1) * 512],
                start=True, stop=True,
            )
        # expbm DMA prefetch first, then exp -> bf16, then mask/bias multiply
        mA = g.work_pool.tile([P, QC], bf, tag="m", bufs=8, name=f"mA{b}_{qc}_{ks}")
        nc.sync.dma_start(
            mA[:], io["expbm"][b, 0, ks * P:(ks + 1) * P, qc * QC:(qc + 1) * QC]
        )
        mB = g.work_pool.tile([P, QC], bf, tag="m", bufs=8, name=f"mB{b}_{qc}_{ks}")
        nc.sync.dma_start(
            mB[:], io["expbm"][b, 1, ks * P:(ks + 1) * P, qc * QC:(qc + 1) * QC]
        )
        eA = g.work_pool.tile([P, QC], bf, tag="e", bufs=6, name=f"eA{b}_{qc}_{ks}")
        nc.scalar.activation(eA[:], ps_sA[:], mybir.ActivationFunctionType.Exp)
        pA = g.work_pool.tile([P, QC], bf, tag="p", bufs=6, name=f"pA{b}_{qc}_{ks}")
        nc.vector.tensor_mul(pA[:], eA[:], mA[:])
        eB = g.work_pool.tile([P, QC], bf, tag="e", bufs=6, name=f"eB{b}_{qc}_{ks}")
        nc.scalar.activation(eB[:], ps_sB[:], mybir.ActivationFunctionType.Exp)
        pB = g.work_pool.tile([P, QC], bf, tag="p", bufs=6, name=f"pB{b}_{qc}_{ks}")
        nc.vector.tensor_mul(pB[:], eB[:], mB[:])

        first = ks == 0
        last = ks == NKS - 1
        for ch in range(QC // 512):
            sl = ts(ch, 512)
            # PV with ones-augmented V: lhsT = [v_h | 1] (M=65); rows 0:64 =
            # OT_h, row 64 = softmax denominator — no separate d matmuls.
            nc.tensor.matmul(
                ps_oA[0:DK + 1, sl], g.v_sb[:, vt, 0:DK + 1], pA[:, sl],
                start=first, stop=last,
            )
            nc.tensor.matmul(
                ps_oB[0:DK + 1, sl], g.v_sb[:, vt, DK + 1:DKC + 2], pB[:, sl],
                start=first, stop=last,
            )
    # normalize: OTn_h = OT_h * (1/d_h). Reciprocal runs on the full
    # base-0 [65, QC] tile (row 64 = 1/d); the 1/d row is broadcast across
    # 64 partitions with a K=1 matmul whose operands both sit at base 64.
    ot_sb = g.work_pool.tile([P, QC], bf, tag="ot", bufs=2, name=f"ot{b}_{qc}")
    otB_t = g.work_pool.tile([DK, QC], bf, tag="otB", bufs=2, name=f"otB{b}_{qc}")
    for hi, (ps_oX, out_ap) in enumerate(((ps_oA, ot_sb[0:DK, :]),
                                          (ps_oB, otB_t[:]))):
        r65 = g.work_pool.tile([65, QC], f32, tag="r65", bufs=2,
                               name=f"r65_{b}_{qc}_{hi}")
        nc.vector.reciprocal_approx_fast(r65[:], ps_oX[0:65, :])
        r65b = g.work_pool.tile([65, QC], bf, tag="r65b", bufs=2,
                                name=f"r65b_{b}_{qc}_{hi}")
        nc.vector.tensor_copy(r65b[:], r65[:])
        ps_r = g.psum_pool.tile([DK, QC], f32, tag="ps_s", bufs=2,
                                name=f"psr{b}_{qc}_{hi}")
        for ch in range(QC // 512):
            nc.tensor.matmul(
                ps_r[:, ts(ch, 512)],
                g.ones65_sb[DK:DK + 1, :],
                r65b[DK:DK + 1, ts(ch, 512)],
                start=True, stop=True,
            )
        rb_sb = g.work_pool.tile([DK, QC], f32, tag="rbs", bufs=2,
                                 name=f"rbs{b}_{qc}_{hi}")
        nc.vector.tensor_copy(rb_sb[:], ps_r[:])
        nc.vector.tensor_mul(out_ap, ps_oX[0:DK, :], rb_sb[:])
    nc.gpsimd.dma_start(ot_sb[DK:P, :], otB_t[:])
    return ot_sb


def _yproj_chunk(nc, g, b, qc, ot_sb):
    """Output projection y[q, :] = OTn[:, q].T @ WoT — emitted deferred so
    these matmuls act as PE filler during later attention chunks."""
    io = g.io
    qs = b * S + qc * QC
    for qsub in range(QC // P):
        ps_y = g.psum_pool.tile([P, QC], f32, tag="ps_d", bufs=1,
                                name=f"psy{b}_{qc}_{qsub}")
        for ch in range(D // 512):
            nc.tensor.matmul(
                ps_y[:, ts(ch, 512)],
                ot_sb[:, ts(qsub, P)],
                g.wo_sb[:, ts(ch, 512)],
                start=True, stop=True,
            )
        y_sb = g.work_pool.tile([P, D], f32, tag="ysb", bufs=2,
                                name=f"ysb{b}_{qc}_{qsub}")
        if qsub % 2 == 0:
            nc.scalar.copy(y_sb[:], ps_y[:])
        else:
            nc.vector.tensor_copy(y_sb[:], ps_y[:])
        nc.sync.dma_start(io["y"][qs + qsub * P:qs + (qsub + 1) * P, :], y_sb[:])


def _build_body(nc, tc, io):
    from contextlib import ExitStack
    ctx = ExitStack()
    g = _Ctx()
    g.io = io
    g.const_pool = ctx.enter_context(tc.tile_pool(name="const", bufs=1))
    g.stream_pool = ctx.enter_context(tc.tile_pool(name="stream", bufs=3))
    g.work_pool = ctx.enter_context(tc.tile_pool(name="work", bufs=2))
    g.psum_pool = ctx.enter_context(tc.tile_pool(name="psum", bufs=2, space="PSUM"))

    # ---- persistent SBUF tensors ----
    g.wq_sb = g.const_pool.tile([P, KO, DKC], bf, tag="wq", name="wq_sb")
    nc.sync.dma_start(g.wq_sb[:], io["wqT"].rearrange("(ko ki) m -> ki ko m", ki=P))
    g.wk_sb = g.const_pool.tile([P, KO, DKC], bf, tag="wk", name="wk_sb")
    nc.sync.dma_start(g.wk_sb[:], io["wkT"].rearrange("(ko ki) m -> ki ko m", ki=P))
    g.wv_sb = g.const_pool.tile([P, KO, DKC], bf, tag="wv", name="wv_sb")
    nc.sync.dma_start(g.wv_sb[:], io["wvT"].rearrange("(ko ki) m -> ki ko m", ki=P))
    g.wo_sb = g.const_pool.tile([P, D], bf, tag="wo", name="wo_sb")
    nc.sync.dma_start(g.wo_sb[:], io["woT"])
    g.ident_sb = g.const_pool.tile([P, P], bf, tag="ident", name="ident_sb")
    make_identity(nc, g.ident_sb[:])
    g.ones65_sb = g.const_pool.tile([65, DK], bf, tag="ones65", name="ones65_sb")
    nc.vector.memset(g.ones65_sb[:], 1.0)

    g.qT_sb = g.const_pool.tile([P, T], bf, tag="qT", name="qT_sb")
    g.kT_sb = g.const_pool.tile([P, T], bf, tag="kT", name="kT_sb")
    # v layout per 128-token tile: [vA(64) | 1 | vB(64) | 1] for ones-aug PV.
    # Full-tile memset; the v copies overwrite all but the ones-columns.
    g.v_sb = g.const_pool.tile([P, T // P, DKC + 2], bf, tag="v", name="v_sb")
    nc.vector.memset(g.v_sb[:], 1.0)

    # batch-0 projections
    for c in range(4):
        _proj_chunk(nc, g, c)
    # batch-0 attention, with batch-1 projections interleaved as PE filler
    ot00 = _attention_chunk(nc, g, 0, 0)
    _yproj_chunk(nc, g, 0, 0, ot00)
    _proj_chunk(nc, g, 4)
    _proj_chunk(nc, g, 5)
    ot01 = _attention_chunk(nc, g, 0, 1)
    _yproj_chunk(nc, g, 0, 1, ot01)
    _proj_chunk(nc, g, 6)
    _proj_chunk(nc, g, 7)
    ot10 = _attention_chunk(nc, g, 1, 0)
    _yproj_chunk(nc, g, 1, 0, ot10)
    ot11 = _attention_chunk(nc, g, 1, 1)
    _yproj_chunk(nc, g, 1, 1, ot11)

    ctx.close()


def build_nc():
    nc = bacc.Bacc("TRN2", target_bir_lowering=False, debug=False,
                   num_devices=N_CORES)
    io = {
        "xqT": nc.dram_tensor("xqT", [D, T], bf, kind="ExternalInput").ap(),
        "xkT": nc.dram_tensor("xkT", [D, T], bf, kind="ExternalInput").ap(),
        "xvT": nc.dram_tensor("xvT", [D, T], bf, kind="ExternalInput").ap(),
        "wqT": nc.dram_tensor("wqT", [D, DKC], bf, kind="ExternalInput").ap(),
        "wkT": nc.dram_tensor("wkT", [D, DKC], bf, kind="ExternalInput").ap(),
        "wvT": nc.dram_tensor("wvT", [D, DKC], bf, kind="ExternalInput").ap(),
        "woT": nc.dram_tensor("woT", [DKC, D], bf, kind="ExternalInput").ap(),
        "expbm": nc.dram_tensor("expbm", [B, HC, S, S], bf,
                                kind="ExternalInput").ap(),
        "y": nc.dram_tensor("y", [T, D], f32, kind="ExternalOutput").ap(),
    }
    with tile.TileContext(nc) as tc:
        _build_body(nc, tc, io)
    nc.compile()
    return nc


_NC_CACHE = None


def _get_nc():
    global _NC_CACHE
    if _NC_CACHE is None:
        _NC_CACHE = build_nc()
    return _NC_CACHE


def make_in_maps(query, key, value, mask, rel_pos_bias,
                 Wq, bq, Wk, bk, Wv, bv, Wo, bo):
    """Host-side sharding/preprocessing -> per-core input dicts."""
    xqT = np.ascontiguousarray(query.reshape(T, D).T.astype(BF16))
    xkT = np.ascontiguousarray(key.reshape(T, D).T.astype(BF16))
    xvT = np.ascontiguousarray(value.reshape(T, D).T.astype(BF16))

    scale = 1.0 / np.sqrt(np.float32(DK))
    maskinv = (~mask[:, 0]).astype(np.float32)          # [B, Sq, Sk]

    # bq/bk handling: scores_full = (q+bq)(k+bk)^T * scale.
    # The (q'+bq)·bk term varies only along q => softmax-invariant, dropped.
    # The bq·(k'+bk) term varies along k; fold exp(delta_k) into expbm when
    # bq is nonzero (needs host k-projection).
    need_delta = bool(np.any(bq))
    if need_delta:
        k_proj = key.reshape(T, D).astype(np.float32) @ Wk.T.astype(np.float32) + bk

    in_maps = []
    for c in range(N_CORES):
        hs = slice(c * DKC, (c + 1) * DKC)
        wqT = np.ascontiguousarray((Wq[hs, :] * scale).T.astype(BF16))
        wkT = np.ascontiguousarray(Wk[hs, :].T.astype(BF16))
        wvT = np.ascontiguousarray(Wv[hs, :].T.astype(BF16))
        woT = np.ascontiguousarray(Wo[:, hs].T.astype(BF16))
        expbm = np.empty((B, HC, S, S), dtype=BF16)
        for hi in range(HC):
            h = c * HC + hi
            ebT = np.exp(rel_pos_bias[0, h].astype(np.float32)).T  # [k, q]
            if need_delta:
                delta = scale * (
                    k_proj[:, h * DK:(h + 1) * DK] @ bq[h * DK:(h + 1) * DK]
                    + np.dot(bq[h * DK:(h + 1) * DK], bk[h * DK:(h + 1) * DK])
                )  # [T] along k
                for b_ in range(B):
                    ebb = ebT * np.exp(delta[b_ * S:(b_ + 1) * S])[:, None]
                    expbm[b_, hi] = (ebb * maskinv[b_].T).astype(BF16)
            else:
                for b_ in range(B):
                    expbm[b_, hi] = (ebT * maskinv[b_].T).astype(BF16)
        in_maps.append({
            "xqT": xqT, "xkT": xkT, "xvT": xvT,
            "wqT": wqT, "wkT": wkT, "wvT": wvT, "woT": woT,
            "expbm": expbm,
        })
    return in_maps


def assemble_output(results, value_bias, Wo, bo):
    out = np.zeros((T, D), np.float32)
    for r in results:
        out += r["y"]
    # exact bv contribution: softmax rows sum to 1 => attn_out += bv,
    # so y += bv @ Wo^T; plus bo.
    out += value_bias.astype(np.float32) @ Wo.T.astype(np.float32)
    out += bo.astype(np.float32)[None, :]
    return out.reshape(B, S, D)


def kernel(query, key, value, mask, rel_pos_bias,
           Wq, bq, Wk, bk, Wv, bv, Wo, bo, _run_kwargs=None):
    query = np.asarray(query); key = np.asarray(key); value = np.asarray(value)
    mask = np.asarray(mask); rel_pos_bias = np.asarray(rel_pos_bias)
    Wq = np.asarray(Wq); Wk = np.asarray(Wk); Wv = np.asarray(Wv)
    Wo = np.asarray(Wo)
    bq = np.asarray(bq); bk = np.asarray(bk); bv = np.asarray(bv)
    bo = np.asarray(bo)

    nc = _get_nc()
    in_maps = make_in_maps(query, key, value, mask, rel_pos_bias,
                           Wq, bq, Wk, bk, Wv, bv, Wo, bo)
    kw = _run_kwargs or {}
    res = run_bass_kernel_spmd(nc, in_maps, core_ids=list(range(N_CORES)), **kw)
    out = assemble_output(res.results, bv, Wo, bo)
    if _run_kwargs is not None:
        kernel._last_results = res
    return out

